# revision 1
# baseline (speedup 1.0000x reference)
"""MoE (top-2 of 8 experts, SwiGLU) Trainium2 kernel, expert-parallel over 8 cores.

Contract: kernel(**inputs) takes the FULL unsharded inputs
  x [2,2048,1024] f32, gate_w [8,1024] f32,
  w1 [8,2048,1024] f32, w2 [8,1024,2048] f32, w3 [8,2048,1024] f32
and returns the FULL output [2,2048,1024] f32.

Strategy (expert-parallel, per the hint "replicate the gate and all-to-all the
token dispatch"): routing (gate softmax + top-2) is computed on host; tokens
are dispatched (gathered) per expert; core e runs the SwiGLU FFN of expert e
over its ~N*TOPK/E assigned tokens (padded to capacity C), pre-scaled by the
combine weight; the host scatter-adds the two expert contributions per token.

Device kernel (per core, feature-major layout so no on-device transposes;
fp32r matmuls = full PE rate at moving-dim >= 256, ~1.5e-4 matmul rel err):
  h1T = w1 @ xg^T   [H, C]   (lhsT = w1T block, rhs = xgT)
  h3T = w3 @ xg^T   [H, C]
  aT  = silu(h1T) * h3T      (ACT Silu + DVE mul, PSUM->SBUF)
  yT  = (w2 @ aT) * combine  [D, C]  (DVE mul on PSUM eviction)

Tokens are processed in free-dim chunks of 256..512 (PSUM-bank bound is 512
fp32; fp32r drops to 1/4 rate below 256), sized so the padded capacity C
hugs the max per-expert token count.
"""

import math
import sys

import numpy as np

for _p in ("/opt/trn_rl_repo", "/opt/pypackages"):
    if _p not in sys.path:
        sys.path.append(_p)

import concourse.bass as bass  # noqa: E402
import concourse.tile as tile  # noqa: E402
from concourse import bacc, mybir  # noqa: E402
from concourse.bass_utils import run_bass_kernel_spmd  # noqa: E402

B, T, D, H, E, TOPK = 2, 2048, 1024, 2048, 8, 2
N = B * T
P = 128
KD = D // P   # 8  k-tiles over D
KH = H // P   # 16 k-tiles over H
HB = H // P   # 16 h blocks of 128 (M dim, stage A)
DB = D // P   # 8  d blocks of 128 (M dim, stage B)

F32 = mybir.dt.float32
F32R = mybir.dt.float32r

# set by test.py to capture an NTFF profile; kernel() stores results here
TRACE = False
TRACE_ALL_CORES = False
LAST_RESULTS = None

_program_cache = {}

# CoreSim doesn't implement Silu; simcheck.py overrides this to Sigmoid.
_ACT_FUNC = mybir.ActivationFunctionType.Silu


# Max tokens per expert handled on host when the count barely exceeds a
# 512 multiple (capacity-factor overflow): full 512-wide chunks minimize the
# per-matmul dispatch overhead (768 vs 1152 MMs for cmax ~1071).
OVERFLOW_MAX = 64


def _chunk_plan(cmax: int) -> list[int]:
    """Token-chunk sizes for the device capacity: each <=512 (PSUM bank),
    as equal as possible (keeps every chunk >=256 for full-rate fp32r when
    cmax allows), 32-aligned, minimal total padding. If cmax is within
    OVERFLOW_MAX above a 512 multiple, use full 512 chunks and let the
    caller route the overflow tokens to the host FFN."""
    if cmax >= 512 and cmax - (cmax // 512) * 512 <= OVERFLOW_MAX:
        return [512] * (cmax // 512)
    n = max(1, math.ceil(cmax / 512))
    chunks = []
    rem = cmax
    for i in range(n):
        s = math.ceil(rem / (n - i) / 32) * 32
        s = min(max(s, 256), 512)
        chunks.append(s)
        rem -= s
    return chunks


def _host_ffn(x_rows, w1e, w2e, w3e, wts):
    """Exact host-side SwiGLU FFN for capacity-overflow tokens (<=64/expert)."""
    h1 = x_rows @ w1e.T
    h3 = x_rows @ w3e.T
    a = h1 / (1.0 + np.exp(-h1)) * h3
    return (a @ w2e.T) * wts[:, None]


def _build_program(chunks: list[int]):
    """Bass program for one core: expert FFN over C = sum(chunks) tokens."""
    C = sum(chunks)
    offs = [sum(chunks[:i]) for i in range(len(chunks))]
    tsls = [bass.ds(o, s) for o, s in zip(offs, chunks)]
    nt = len(chunks)

    nc = bacc.Bacc(
        "TRN2", target_bir_lowering=False, debug=False,
        enable_asserts=False, num_devices=8,
    )
    xgT_d = nc.dram_tensor("xgT", [D, C], F32R, kind="ExternalInput").ap()
    w1T_d = nc.dram_tensor("w1T", [D, H], F32R, kind="ExternalInput").ap()
    w3T_d = nc.dram_tensor("w3T", [D, H], F32R, kind="ExternalInput").ap()
    w2T_d = nc.dram_tensor("w2T", [H, D], F32R, kind="ExternalInput").ap()
    scl_d = nc.dram_tensor("scale_b", [P, C], F32, kind="ExternalInput").ap()
    yT_d = nc.dram_tensor("yT", [D, C], F32, kind="ExternalOutput").ap()

    # DRAM views with the 128-partition k-tile split exposed
    xgT_v = xgT_d.rearrange("(k p) c -> p k c", p=P)     # [P, KD, C]
    w1T_v = w1T_d.rearrange("(k p) h -> p k h", p=P)     # [P, KD, H]
    w3T_v = w3T_d.rearrange("(k p) h -> p k h", p=P)
    w2T_v = w2T_d.rearrange("(k p) d -> p k d", p=P)     # [P, KH, D]

    with tile.TileContext(nc) as tc:
        with tc.tile_pool(name="resident", bufs=1) as res_pool, \
             tc.tile_pool(name="w13", bufs=3) as w13_pool, \
             tc.tile_pool(name="w2", bufs=3) as w2_pool, \
             tc.tile_pool(name="ev", bufs=3) as ev_pool, \
             tc.tile_pool(name="psum", bufs=2, space="PSUM") as ps_pool:

            # xg loaded in (token-chunk, k) slices on the sync (HWDGE) queue
            # so stage A's first psum groups only gate on their own slice,
            # while the w1/w3 stream runs in parallel on the gpsimd queue
            # (one tile per queue: cross-queue writes into a single tile
            # break the DMA->matmul ordering).
            # One 3D DMA per token chunk: chunk granularity is what the
            # matmul deps need (a psum group consumes all 8 k-slices), and
            # fewer dma_starts cut per-transfer overhead on the head stream.
            xg = res_pool.tile([P, KD, C], F32R, tag="xg")
            for t in range(nt):
                nc.sync.dma_start(xg[:, :, tsls[t]], xgT_v[:, :, tsls[t]])
            act = res_pool.tile([P, KH, C], F32R, tag="act")

            # ---- stage A: act[H, C] = silu(w1 @ xgT) * (w3 @ xgT) ----
            # h-blocks processed in pairs with the token-chunk loop outside
            # the pair: two h-blocks of chunk-t compute run before chunk t+1
            # is touched, hiding the next xg chunk's DMA arrival.
            for hp in range(0, HB, 2):
                pair = [h for h in (hp, hp + 1) if h < HB]
                w1ts, w3ts = [], []
                for i, h in enumerate(pair):
                    w1t = w13_pool.tile([P, KD, P], F32R, tag=f"w1_{i}",
                                        bufs=2, name=f"w1t_{h}")
                    nc.gpsimd.dma_start(w1t[:], w1T_v[:, :, h * P:(h + 1) * P])
                    w3t = w13_pool.tile([P, KD, P], F32R, tag=f"w3_{i}",
                                        bufs=2, name=f"w3t_{h}")
                    nc.gpsimd.dma_start(w3t[:], w3T_v[:, :, h * P:(h + 1) * P])
                    w1ts.append(w1t)
                    w3ts.append(w3t)
                for t in range(nt):
                    tsl = tsls[t]
                    for i, h in enumerate(pair):
                        ph1 = ps_pool.tile([P, chunks[t]], F32, tag="h1",
                                           bufs=3, name=f"ph1_{h}_{t}")
                        for k in range(KD):
                            nc.tensor.matmul(ph1[:], w1ts[i][:, k, :],
                                             xg[:, k, tsl],
                                             start=(k == 0), stop=(k == KD - 1))
                        ph3 = ps_pool.tile([P, chunks[t]], F32, tag="h3",
                                           bufs=3, name=f"ph3_{h}_{t}")
                        for k in range(KD):
                            nc.tensor.matmul(ph3[:], w3ts[i][:, k, :],
                                             xg[:, k, tsl],
                                             start=(k == 0), stop=(k == KD - 1))
                        asl = act[:, h, tsl]
                        nc.scalar.activation(asl, ph1[:], func=_ACT_FUNC)
                        nc.vector.tensor_mul(asl, asl, ph3[:])

            # combine-weight row (needed only for stage B evictions)
            scl = res_pool.tile([P, C], F32, tag="scl")
            nc.gpsimd.dma_start(scl[:], scl_d[:, :])

            # ---- stage B: yT[D, C] = (w2 @ act) * scale ----
            for d in range(DB):
                w2t = w2_pool.tile([P, KH, P], F32R, tag="w2")
                nc.sync.dma_start(w2t[:], w2T_v[:, :, d * P:(d + 1) * P])
                for t in range(nt):
                    tsl = tsls[t]
                    py = ps_pool.tile([P, chunks[t]], F32, tag="y")
                    for k in range(KH):
                        nc.tensor.matmul(py[:], w2t[:, k, :], act[:, k, tsl],
                                         start=(k == 0), stop=(k == KH - 1))
                    ysb = ev_pool.tile([P, chunks[t]], F32, tag="ysb")
                    nc.vector.tensor_mul(ysb[:], py[:], scl[:, tsl])
                    nc.scalar.dma_start(yT_d[d * P:(d + 1) * P, tsl], ysb[:])

    nc.compile()
    return nc


def _route(flat, gate_w):
    """Host replica of the reference router. Returns top-2 expert ids and
    combine weights (top-2 of softmax, renormalized)."""
    logits = flat @ gate_w.T                                   # [N, E] f32
    m = logits.max(axis=1, keepdims=True)
    p = np.exp((logits - m).astype(np.float32))
    probs = p / p.sum(axis=1, keepdims=True)
    idx = np.argsort(-probs, axis=1, kind="stable")[:, :TOPK]  # [N, 2]
    top = np.take_along_axis(probs, idx, axis=1)               # [N, 2]
    wn = top / top.sum(axis=1, keepdims=True)
    return idx, wn


def kernel(x, gate_w, w1, w2, w3):
    global LAST_RESULTS
    x = np.asarray(x, np.float32)
    gate_w = np.asarray(gate_w, np.float32)
    w1 = np.asarray(w1, np.float32)
    w2 = np.asarray(w2, np.float32)
    w3 = np.asarray(w3, np.float32)

    flat = x.reshape(N, D)
    idx, wn = _route(flat, gate_w)

    sels, wsels = [], []
    for e in range(E):
        hit = idx == e                                         # [N, 2]
        sel = np.nonzero(hit.any(axis=1))[0]
        k = hit[sel, 1].astype(np.int64)                       # which top slot
        sels.append(sel)
        wsels.append(wn[sel, k])
    cmax = max(len(s) for s in sels)
    chunks = _chunk_plan(cmax)
    C = sum(chunks)

    xT = np.ascontiguousarray(flat.T)                          # [D, N]
    in_maps = []
    for e in range(E):
        sel = sels[e][:C]                  # tokens beyond C go to _host_ffn
        xgT = np.zeros((D, C), np.float32)
        xgT[:, :len(sel)] = xT[:, sel]
        scale_b = np.zeros((P, C), np.float32)
        scale_b[:, :len(sel)] = wsels[e][:C][None, :]
        in_maps.append({
            "xgT": xgT,
            "w1T": np.ascontiguousarray(w1[e].T),
            "w3T": np.ascontiguousarray(w3[e].T),
            "w2T": np.ascontiguousarray(w2[e].T),
            "scale_b": scale_b,
        })

    key = tuple(chunks)
    if key not in _program_cache:
        _program_cache[key] = _build_program(chunks)
    nc = _program_cache[key]

    res = run_bass_kernel_spmd(
        nc, in_maps, core_ids=list(range(E)),
        trace=TRACE,
        trace_cores=list(range(E)) if (TRACE and TRACE_ALL_CORES) else None,
    )
    LAST_RESULTS = res

    out = np.zeros((N, D), np.float32)
    for e in range(E):
        sel = sels[e][:C]
        out[sel] += res.results[e]["yT"][:, :len(sel)].T
        over = sels[e][C:]
        if len(over):
            out[over] += _host_ffn(flat[over], w1[e], w2[e], w3[e],
                                   wsels[e][C:])
    return out.reshape(B, T, D)



# revision 2
# speedup vs baseline: 1.2339x; 1.2339x over previous
"""MoE (top-2 of 8 experts, SwiGLU) Trainium2 kernel, expert-parallel over 8 cores.

Contract: kernel(**inputs) takes the FULL unsharded inputs
  x [2,2048,1024] f32, gate_w [8,1024] f32,
  w1 [8,2048,1024] f32, w2 [8,1024,2048] f32, w3 [8,2048,1024] f32
and returns the FULL output [2,2048,1024] f32.

Strategy (expert-parallel, per the hint "replicate the gate and all-to-all the
token dispatch"): routing (gate softmax + top-2) is computed on host; tokens
are dispatched (gathered) per expert; core e runs the SwiGLU FFN of expert e
over its ~N*TOPK/E assigned tokens (padded to capacity C), pre-scaled by the
combine weight; the host scatter-adds the two expert contributions per token.

Device kernel (per core, feature-major layout so no on-device transposes).
All matmul operands are bf16 (same 1 cycle/row PE rate as fp32r at 512-wide
moving dim, but half the DMA bytes, half the LDWEIGHTS time so weight loads
fully hide under the matmul stream, and half the SBUF footprint; fp32 PSUM
accumulate keeps rel err ~4e-3):
  h1T = w1 @ xg^T   [H, C]   (lhsT = w1T block, rhs = xgT)
  h3T = w3 @ xg^T   [H, C]
  aT  = silu(h1T) * h3T      (ACT Silu + DVE mul, PSUM->SBUF, bf16)
  yT  = (w2 @ aT) * combine  [D, C]  (DVE mul on PSUM eviction, f32 out)

Tokens are processed in free-dim chunks of <=512 (PSUM-bank bound). DMA is
spread over the three DGE queues (sync/scalar HWDGE + gpsimd SWDGE) so the
first chunk of xg and the first h-block's weights arrive in parallel, which
sets the time-to-first-matmul.
"""

import math
import sys

import numpy as np

for _p in ("/opt/trn_rl_repo", "/opt/pypackages"):
    if _p not in sys.path:
        sys.path.append(_p)

import ml_dtypes  # noqa: E402

import concourse.bass as bass  # noqa: E402
import concourse.tile as tile  # noqa: E402
from concourse import bacc, mybir  # noqa: E402
from concourse.bass_utils import run_bass_kernel_spmd  # noqa: E402

B, T, D, H, E, TOPK = 2, 2048, 1024, 2048, 8, 2
N = B * T
P = 128
KD = D // P   # 8  k-tiles over D
KD2 = KD // 2  # 4 k-tiles per xg half tile
KH = H // P   # 16 k-tiles over H
HB = H // P   # 16 h blocks of 128 (M dim, stage A)
DB = D // P   # 8  d blocks of 128 (M dim, stage B)

F32 = mybir.dt.float32
BF16 = mybir.dt.bfloat16
NP_BF16 = ml_dtypes.bfloat16

# set by test.py to capture an NTFF profile; kernel() stores results here
TRACE = False
TRACE_ALL_CORES = False
LAST_RESULTS = None

_program_cache = {}

# CoreSim doesn't implement Silu; simcheck.py overrides this to Sigmoid.
_ACT_FUNC = mybir.ActivationFunctionType.Silu


# Max tokens per expert handled on host when the count barely exceeds a
# 512 multiple (capacity-factor overflow): full 512-wide chunks minimize the
# per-matmul dispatch overhead.
OVERFLOW_MAX = 64


def _chunk_plan(cmax: int) -> list[int]:
    """Token-chunk sizes for the device capacity: each <=512 (PSUM bank),
    as equal as possible, 32-aligned, minimal total padding. If cmax is
    within OVERFLOW_MAX above a 512 multiple, use full 512 chunks and let
    the caller route the overflow tokens to the host FFN."""
    if cmax >= 512 and cmax - (cmax // 512) * 512 <= OVERFLOW_MAX:
        return [512] * (cmax // 512)
    n = max(1, math.ceil(cmax / 512))
    chunks = []
    rem = cmax
    for i in range(n):
        s = math.ceil(rem / (n - i) / 32) * 32
        s = min(max(s, 256), 512)
        chunks.append(s)
        rem -= s
    return chunks


def _host_ffn(x_rows, w1e, w2e, w3e, wts):
    """Host-side SwiGLU FFN for capacity-overflow tokens (<=64/expert),
    in bf16 operand precision to match the device kernel."""
    xb = x_rows.astype(NP_BF16).astype(np.float32)
    h1 = xb @ w1e.astype(NP_BF16).astype(np.float32).T
    h3 = xb @ w3e.astype(NP_BF16).astype(np.float32).T
    a = h1 / (1.0 + np.exp(-h1)) * h3
    a = a.astype(NP_BF16).astype(np.float32)
    return (a @ w2e.astype(NP_BF16).astype(np.float32).T) * wts[:, None]


def _build_program(chunks: list[int]):
    """Bass program for one core: expert FFN over C = sum(chunks) tokens."""
    C = sum(chunks)
    offs = [sum(chunks[:i]) for i in range(len(chunks))]
    tsls = [bass.ds(o, s) for o, s in zip(offs, chunks)]
    nt = len(chunks)

    nc = bacc.Bacc(
        "TRN2", target_bir_lowering=False, debug=False,
        enable_asserts=False, num_devices=8,
    )
    xgT_d = nc.dram_tensor("xgT", [D, C], BF16, kind="ExternalInput").ap()
    w1T_d = nc.dram_tensor("w1T", [D, H], BF16, kind="ExternalInput").ap()
    w3T_d = nc.dram_tensor("w3T", [D, H], BF16, kind="ExternalInput").ap()
    w2T_d = nc.dram_tensor("w2T", [H, D], BF16, kind="ExternalInput").ap()
    scl_d = nc.dram_tensor("scale_b", [P, C], F32, kind="ExternalInput").ap()
    yT_d = nc.dram_tensor("yT", [D, C], F32, kind="ExternalOutput").ap()

    # DRAM views with the 128-partition k-tile split exposed
    xgT_v = xgT_d.rearrange("(k p) c -> p k c", p=P)     # [P, KD, C]
    w1T_v = w1T_d.rearrange("(k p) h -> p k h", p=P)     # [P, KD, H]
    w3T_v = w3T_d.rearrange("(k p) h -> p k h", p=P)
    w2T_v = w2T_d.rearrange("(k p) d -> p k d", p=P)     # [P, KH, D]

    with tile.TileContext(nc) as tc:
        with tc.tile_pool(name="resident", bufs=1) as res_pool, \
             tc.tile_pool(name="w13", bufs=3) as w13_pool, \
             tc.tile_pool(name="w2", bufs=3) as w2_pool, \
             tc.tile_pool(name="ev", bufs=3) as ev_pool, \
             tc.tile_pool(name="psum", bufs=2, space="PSUM") as ps_pool:

            # xg split into two half-K tiles, each owned by ONE DGE queue
            # (a tile written from two queues breaks DMA->matmul ordering):
            # k 0..3 on sync, k 4..7 on scalar, streamed chunk 0 first so
            # the first psum group's gate is ~0.5 MB per queue.
            xg_lo = res_pool.tile([P, KD2, C], BF16, tag="xg_lo")
            xg_hi = res_pool.tile([P, KD2, C], BF16, tag="xg_hi")
            for t in range(nt):
                nc.sync.dma_start(xg_lo[:, :, tsls[t]],
                                  xgT_v[:, 0:KD2, tsls[t]])
                nc.scalar.dma_start(xg_hi[:, :, tsls[t]],
                                    xgT_v[:, KD2:KD, tsls[t]])
            act = res_pool.tile([P, KH, C], BF16, tag="act")

            # ---- stage A: act[H, C] = silu(w1 @ xgT) * (w3 @ xgT) ----
            # h-blocks processed in pairs with the token-chunk loop outside
            # the pair: two h-blocks of chunk-t compute run before chunk t+1
            # is touched, hiding the next xg chunk's DMA arrival. The first
            # pair's w1+w3 go on gpsimd (independent of the xg queues);
            # later pairs stream w1 on gpsimd and w3 on sync (in-order
            # behind xg, which has fully issued by then).
            for hp in range(0, HB, 2):
                pair = [h for h in (hp, hp + 1) if h < HB]
                w1ts, w3ts = [], []
                for i, h in enumerate(pair):
                    w1t = w13_pool.tile([P, KD, P], BF16, tag=f"w1_{i}",
                                        bufs=2, name=f"w1t_{h}")
                    nc.gpsimd.dma_start(w1t[:], w1T_v[:, :, h * P:(h + 1) * P])
                    w3t = w13_pool.tile([P, KD, P], BF16, tag=f"w3_{i}",
                                        bufs=2, name=f"w3t_{h}")
                    w3q = nc.gpsimd if hp == 0 else nc.sync
                    w3q.dma_start(w3t[:], w3T_v[:, :, h * P:(h + 1) * P])
                    w1ts.append(w1t)
                    w3ts.append(w3t)
                for t in range(nt):
                    tsl = tsls[t]
                    for i, h in enumerate(pair):
                        ph1 = ps_pool.tile([P, chunks[t]], F32, tag="h1",
                                           bufs=3, name=f"ph1_{h}_{t}")
                        for k in range(KD2):
                            nc.tensor.matmul(ph1[:], w1ts[i][:, k, :],
                                             xg_lo[:, k, tsl],
                                             start=(k == 0), stop=False)
                        for k in range(KD2):
                            nc.tensor.matmul(ph1[:], w1ts[i][:, KD2 + k, :],
                                             xg_hi[:, k, tsl],
                                             start=False, stop=(k == KD2 - 1))
                        ph3 = ps_pool.tile([P, chunks[t]], F32, tag="h3",
                                           bufs=3, name=f"ph3_{h}_{t}")
                        for k in range(KD2):
                            nc.tensor.matmul(ph3[:], w3ts[i][:, k, :],
                                             xg_lo[:, k, tsl],
                                             start=(k == 0), stop=False)
                        for k in range(KD2):
                            nc.tensor.matmul(ph3[:], w3ts[i][:, KD2 + k, :],
                                             xg_hi[:, k, tsl],
                                             start=False, stop=(k == KD2 - 1))
                        asl = act[:, h, tsl]
                        nc.scalar.activation(asl, ph1[:], func=_ACT_FUNC)
                        nc.vector.tensor_mul(asl, asl, ph3[:])

            # combine-weight row (needed only for stage B evictions)
            scl = res_pool.tile([P, C], F32, tag="scl")
            nc.gpsimd.dma_start(scl[:], scl_d[:, :])

            # ---- stage B: yT[D, C] = (w2 @ act) * scale ----
            for d in range(DB):
                w2t = w2_pool.tile([P, KH, P], BF16, tag="w2")
                nc.sync.dma_start(w2t[:], w2T_v[:, :, d * P:(d + 1) * P])
                for t in range(nt):
                    tsl = tsls[t]
                    py = ps_pool.tile([P, chunks[t]], F32, tag="y")
                    for k in range(KH):
                        nc.tensor.matmul(py[:], w2t[:, k, :], act[:, k, tsl],
                                         start=(k == 0), stop=(k == KH - 1))
                    ysb = ev_pool.tile([P, chunks[t]], F32, tag="ysb")
                    nc.vector.tensor_mul(ysb[:], py[:], scl[:, tsl])
                    nc.scalar.dma_start(yT_d[d * P:(d + 1) * P, tsl], ysb[:])

    nc.compile()
    return nc


def _route(flat, gate_w):
    """Host replica of the reference router. Returns top-2 expert ids and
    combine weights (top-2 of softmax, renormalized)."""
    logits = flat @ gate_w.T                                   # [N, E] f32
    m = logits.max(axis=1, keepdims=True)
    p = np.exp((logits - m).astype(np.float32))
    probs = p / p.sum(axis=1, keepdims=True)
    idx = np.argsort(-probs, axis=1, kind="stable")[:, :TOPK]  # [N, 2]
    top = np.take_along_axis(probs, idx, axis=1)               # [N, 2]
    wn = top / top.sum(axis=1, keepdims=True)
    return idx, wn


def kernel(x, gate_w, w1, w2, w3):
    global LAST_RESULTS
    x = np.asarray(x, np.float32)
    gate_w = np.asarray(gate_w, np.float32)
    w1 = np.asarray(w1, np.float32)
    w2 = np.asarray(w2, np.float32)
    w3 = np.asarray(w3, np.float32)

    flat = x.reshape(N, D)
    idx, wn = _route(flat, gate_w)

    sels, wsels = [], []
    for e in range(E):
        hit = idx == e                                         # [N, 2]
        sel = np.nonzero(hit.any(axis=1))[0]
        k = hit[sel, 1].astype(np.int64)                       # which top slot
        sels.append(sel)
        wsels.append(wn[sel, k])
    cmax = max(len(s) for s in sels)
    chunks = _chunk_plan(cmax)
    C = sum(chunks)

    xT = np.ascontiguousarray(flat.T)                          # [D, N]
    in_maps = []
    for e in range(E):
        sel = sels[e][:C]                  # tokens beyond C go to _host_ffn
        xgT = np.zeros((D, C), NP_BF16)
        xgT[:, :len(sel)] = xT[:, sel].astype(NP_BF16)
        scale_b = np.zeros((P, C), np.float32)
        scale_b[:, :len(sel)] = wsels[e][:C][None, :]
        in_maps.append({
            "xgT": xgT,
            "w1T": np.ascontiguousarray(w1[e].T.astype(NP_BF16)),
            "w3T": np.ascontiguousarray(w3[e].T.astype(NP_BF16)),
            "w2T": np.ascontiguousarray(w2[e].T.astype(NP_BF16)),
            "scale_b": scale_b,
        })

    key = tuple(chunks)
    if key not in _program_cache:
        _program_cache[key] = _build_program(chunks)
    nc = _program_cache[key]

    res = run_bass_kernel_spmd(
        nc, in_maps, core_ids=list(range(E)),
        trace=TRACE,
        trace_cores=list(range(E)) if (TRACE and TRACE_ALL_CORES) else None,
    )
    LAST_RESULTS = res

    out = np.zeros((N, D), np.float32)
    for e in range(E):
        sel = sels[e][:C]
        out[sel] += res.results[e]["yT"][:, :len(sel)].T
        over = sels[e][C:]
        if len(over):
            out[over] += _host_ffn(flat[over], w1[e], w2[e], w3[e],
                                   wsels[e][C:])
    return out.reshape(B, T, D)
